# revision 4
# baseline (speedup 1.0000x reference)
"""Multi-head attention (B=4, S=2048, D=1024, H=16, causal) on 8 Trainium2
NeuronCores via Bass/Tile — two SPMD launches.

  L1  QKV projections, row-sharded: core c computes (x @ W.T + b)^T for its
      1/8 of the B*S rows, output in [outcol, rows] (transposed) bf16 layout.
      Loops are ordered kt-outer / oc-inner over 512-row chunks so the first
      matmul issues after ~0.4 MB of DMA instead of 12.6 MB.

  L2  Fused attention + output projection.  Work unit = (batch, 512-row
      q-block) x 8 heads.  Every core runs the SAME program over four q-block
      slots (j = 0, 3, 1, 2 with 4/16/8/12 causal k-tiles); which heads a
      slot covers differs per core only in the DATA the host packs:
        even core (b = c//2): heads 0-7 on j in {0,3}, heads 8-15 on {1,2}
        odd  core           : heads 8-15 on j in {0,3}, heads 0-7 on {1,2}
      Scores are computed transposed (ST = K @ Q^T) so the softmax sum runs
      over PSUM partitions via a ones-column appended to V.  Causal diagonal
      tiles stream only their valid column spans (scores, exp, AV all
      trimmed); the 128x128 boundary strip is a post-exp 0/1 multiply on
      GPSIMD.  Softmax normalization happens on-device (DVE reciprocal +
      GPSIMD partition_broadcast + DVE multiply), heads are stacked in pairs
      on 128 partitions (odd head shifted via SBUF-to-SBUF DMA), and the
      output projection accumulates y^T = Wo^T-tiles @ A^T over the 4
      head-pairs of the slot's head group.  The host sums the two per-head-
      group partials of each q-block and adds the output bias.

Matmul operands are bf16 (1 cycle/row on the PE); accumulation is fp32 in
PSUM.  fp8 was evaluated and rejected: e4m3 anywhere on the value path puts
max-normalized error at 2.5e-2..5e-2 against the 2e-2 gate.
"""

import sys

sys.path.insert(0, "/opt/trn_rl_repo")

import ml_dtypes
import numpy as np

import concourse.bacc as bacc
import concourse.tile as tile
from concourse import mybir
from concourse.bass_utils import run_bass_kernel_spmd

F32 = mybir.dt.float32
BF16 = mybir.dt.bfloat16
NPBF = ml_dtypes.bfloat16
EXP = mybir.ActivationFunctionType.Exp

B, S, D, H, DK = 4, 2048, 1024, 16, 64
NCORES = 8
RPC = B * S // NCORES      # rows per core in the projection launch (1024)
SCALE = 1.0 / np.sqrt(DK)  # folded into the exp activation

# L2 slot table: slot -> (head-pair group, q-block j).  Tile counts 4/16/8/12.
SLOT_GJ = [(0, 0), (0, 3), (1, 1), (1, 2)]
SLOT_NT = [4 * j + 4 for _, j in SLOT_GJ]
SLOT_QCOL = [0, 512, 0, 512]       # column of the slot's q-block in qt_d[hp]
SLOT_ORDER = [1, 3, 2, 0]          # process big slots first, smallest last
GRP_KL = [2048, 1536]              # k columns needed by head-pair group
GRP_NT = [16, 12]                  # vp k-tiles needed by head-pair group

_CACHE = {}


def _build_proj():
    """L1: yT = (x @ W.T + b)^T for q/k/v, one 1024-row shard."""
    nc = bacc.Bacc(trn_type="TRN2", target_bir_lowering=False)
    ins, outs = {}, {}
    for p in ("q", "k", "v"):
        ins[p] = (
            nc.dram_tensor(f"x{p}", [D, RPC], BF16, kind="ExternalInput"),
            nc.dram_tensor(f"w{p}", [D, D], BF16, kind="ExternalInput"),
            nc.dram_tensor(f"b{p}", [128, D // 128], F32, kind="ExternalInput"),
        )
        outs[p] = nc.dram_tensor(f"{p}t", [D, RPC], BF16, kind="ExternalOutput")

    KT, OCT, CH = D // 128, D // 128, RPC // 512
    with tile.TileContext(nc) as tc:
        with (
            tc.tile_pool(name="big", bufs=2) as big,
            tc.tile_pool(name="bias", bufs=2) as bias,
            tc.tile_pool(name="outp", bufs=6) as outp,
            tc.tile_pool(name="ps", bufs=1, space="PSUM") as psp,
        ):
            wz = bias.tile([128, 512], BF16, tag="wz")
            nc.vector.memset(wz[:], 0.0)
            wp = psp.tile([128, 512], F32, tag="ps0", name="warm")
            for r in range(16):
                nc.tensor.matmul(wp[:], wz[:, 0:128], wz[:], start=(r == 0),
                                 stop=(r == 15))
            for p in ("q", "k", "v"):
                x_d, w_d, b_d = ins[p]
                xt = big.tile([128, KT, RPC], BF16, tag="xt")
                wt = big.tile([128, KT, D], BF16, tag="wt")
                bt = bias.tile([128, OCT], F32, tag="bt")
                for kt in range(KT):
                    nc.sync.dma_start(out=wt[:, kt, :],
                                      in_=w_d[kt * 128:(kt + 1) * 128, :])
                    nc.sync.dma_start(out=xt[:, kt, 0:512],
                                      in_=x_d[kt * 128:(kt + 1) * 128, 0:512])
                nc.sync.dma_start(out=bt[:], in_=b_d[:])
                for kt in range(KT):
                    nc.sync.dma_start(out=xt[:, kt, 512:RPC],
                                      in_=x_d[kt * 128:(kt + 1) * 128, 512:RPC])
                for ch in range(CH):
                    csl = slice(ch * 512, (ch + 1) * 512)
                    pss = []
                    for kt in range(KT):
                        for oc in range(OCT):
                            if kt == 0:
                                pss.append(psp.tile([128, 512], F32,
                                                    tag=f"ps{oc}",
                                                    name=f"ps{p}{ch}_{oc}"))
                            nc.tensor.matmul(
                                pss[oc][:],
                                wt[:, kt, oc * 128:(oc + 1) * 128],
                                xt[:, kt, csl],
                                start=(kt == 0),
                                stop=(kt == KT - 1),
                            )
                    for oc in range(OCT):
                        ob = outp.tile([128, 512], BF16, tag="ob")
                        nc.vector.tensor_scalar_add(ob[:], pss[oc][:],
                                                    bt[:, oc:oc + 1])
                        nc.sync.dma_start(
                            out=outs[p][oc * 128:(oc + 1) * 128, csl],
                            in_=ob[:])
    nc.compile()
    return nc


def _build_attn():
    """L2: fused attention + partial output projection (see module doc).

    Inputs (per core):
      qt  [8, 128, 1024] bf16 : Q^T head-pair tiles; cols [0:512] = the
          slot-jA block, [512:1024] = the slot-jB block of that head pair.
      kt  [8, 128, 2048] bf16 : K^T head-pair tiles (group 1 uses 0:1536).
      vp  [8, 128, 2, 16, 65] bf16 : V per head with a ones column appended,
          vp[hp, p, hh, t, c] = V[head(hp, hh), k = 128 t + p, c].
      wo  [8, 128, 1024] bf16 : Wo^T rows for each head pair.
      m01 [128, 128] bf16 : causal 0/1 boundary strip (valid iff col >= row).
    Output:
      y   [4, 8, 128, 512] f32 : per slot, y^T of-tiles for the slot's head
          group (partial over 8 heads; host sums the two groups).
    """
    nc = bacc.Bacc(trn_type="TRN2", target_bir_lowering=False)
    qt_d = nc.dram_tensor("qt", [8, 128, 1024], BF16, kind="ExternalInput")
    kt_d = nc.dram_tensor("kt", [8, 128, 2048], BF16, kind="ExternalInput")
    vp_d = nc.dram_tensor("vp", [8, 128, 2, 16, DK + 1], BF16,
                          kind="ExternalInput")
    wo_d = nc.dram_tensor("wo", [8, 128, D], BF16, kind="ExternalInput")
    mk_d = nc.dram_tensor("m01", [128, 128], BF16, kind="ExternalInput")
    y_d = nc.dram_tensor("y", [4, 8, 128, 512], F32, kind="ExternalOutput")

    with tile.TileContext(nc) as tc:
        with (
            tc.tile_pool(name="qk", bufs=1) as qk,
            tc.tile_pool(name="epool", bufs=6) as epool,
            tc.tile_pool(name="upool", bufs=2) as upool,
            tc.tile_pool(name="npool", bufs=2) as npool,
            tc.tile_pool(name="ypool", bufs=2) as ypool,
            tc.tile_pool(name="stp", bufs=2, space="PSUM") as stp,
            tc.tile_pool(name="otp", bufs=2, space="PSUM") as otp,
        ):
            qt = qk.tile([128, 8, 1024], BF16, tag="qt")
            kt = qk.tile([128, 8, 2048], BF16, tag="kt")
            vp = qk.tile([128, 8, 2, 16, DK + 1], BF16, tag="vp")
            wo = qk.tile([128, 8, D], BF16, tag="wo")
            mk = qk.tile([128, 128], BF16, tag="mk")
            nc.sync.dma_start(out=mk[:], in_=mk_d[:])
            # hp 0-3 belong to the first-processed slots; load them first.
            for hp in range(8):
                g = hp // 4
                kl, nt = GRP_KL[g], GRP_NT[g]
                nc.sync.dma_start(out=qt[:, hp, :], in_=qt_d[hp])
                nc.sync.dma_start(out=kt[:, hp, 0:512], in_=kt_d[hp, :, 0:512])
                nc.sync.dma_start(out=vp[:, hp, :, 0:nt, :],
                                  in_=vp_d[hp, :, :, 0:nt, :])
                nc.sync.dma_start(out=kt[:, hp, 512:kl],
                                  in_=kt_d[hp, :, 512:kl])
            for hp in range(8):
                nc.sync.dma_start(out=wo[:, hp, :], in_=wo_d[hp])

            # PE p-state warm-up + exp table preload.
            wz = npool.tile([128, 512], BF16, tag="wz")
            nc.vector.memset(wz[:], 0.0)
            wp = stp.tile([128, 1024], F32, tag="st", name="warm")
            for r in range(16):
                nc.tensor.matmul(wp[:, 0:512], wz[:, 0:128], wz[:],
                                 start=(r == 0), stop=(r == 15))
            we = epool.tile([128, 1024], BF16, tag="e", name="warme")
            nc.scalar.activation(we[0:1, 0:8], wp[0:1, 0:8], EXP, scale=1.0)

            u_tiles = {}          # slot -> [u tile per head-pair slot]
            deferred = []         # pending out-proj emitters

            def emit_norm(s, mm, ots):
                """Normalize one head pair: u = ot[0:64] / ot[64] (per q).

                partition_broadcast only works from a partition-0 source on
                HW, and the fast reciprocal is only validated on SBUF@0, so
                the denominators hop: DVE copy (psum@64 -> sbuf@64), DMA
                partition shift to 0, approx reciprocal, broadcast.
                """
                dsb = npool.tile([65, 1024], F32, tag="dsb")
                d0 = npool.tile([1, 1024], F32, tag="d0")
                r0 = npool.tile([1, 1024], F32, tag="r0")
                bc = npool.tile([64, 1024], F32, tag="bc")
                u = upool.tile([128, 512], BF16, tag=f"u{mm}",
                               name=f"u_{s}_{mm}")
                tmp = npool.tile([64, 512], BF16, tag="tmp")
                for hh in range(2):
                    csl = slice(hh * 512, hh * 512 + 512)
                    nc.vector.tensor_copy(dsb[64:65, csl], ots[hh][64:65, :])
                nc.sync.dma_start(out=d0[:], in_=dsb[64:65, :])
                nc.vector.reciprocal_approx_fast(r0[:], d0[:])
                for hh in range(2):
                    csl = slice(hh * 512, hh * 512 + 512)
                    nc.gpsimd.partition_broadcast(bc[0:64, csl],
                                                  r0[0:1, csl])
                nc.vector.tensor_mul(u[0:64, :], ots[0][0:64, :],
                                     bc[0:64, 0:512])
                nc.vector.tensor_mul(tmp[:], ots[1][0:64, :],
                                     bc[0:64, 512:1024])
                nc.sync.dma_start(out=u[64:128, :], in_=tmp[:])
                u_tiles.setdefault(s, [None] * 4)[mm] = u
                if all(t is not None for t in u_tiles[s]):
                    g = SLOT_GJ[s][0]
                    us = u_tiles[s]
                    for ofp in range(4):
                        deferred.append((s, g, ofp, us))

            def emit_outproj():
                if not deferred:
                    return
                s, g, ofp, us = deferred.pop(0)
                y = stp.tile([128, 1024], F32, tag="st", name=f"y_{s}_{ofp}")
                for half in range(2):
                    of = 2 * ofp + half
                    for mm in range(4):
                        nc.tensor.matmul(
                            y[:, half * 512:half * 512 + 512],
                            wo[:, 4 * g + mm, of * 128:(of + 1) * 128],
                            us[mm][:],
                            start=(mm == 0),
                            stop=(mm == 3),
                        )
                ysb = ypool.tile([128, 1024], F32, tag="ysb")
                nc.vector.tensor_copy(ysb[:], y[:])
                for half in range(2):
                    nc.sync.dma_start(out=y_d[s, 2 * ofp + half],
                                      in_=ysb[:, half * 512:half * 512 + 512])

            def emit_av(s, mm, hp, i, n, off, e, ots):
                for hh in range(2):
                    nc.tensor.matmul(
                        ots[hh][:, off:512],
                        vp[:, hp, hh, i, :],
                        e[:, hh * 512 + off:hh * 512 + 512],
                        start=(i == 0),
                        stop=(i == n - 1),
                    )
                if i == n - 1:
                    emit_norm(s, mm, ots)

            pend = None
            ots = None
            for s in SLOT_ORDER:
                g, j = SLOT_GJ[s]
                n, qcol = SLOT_NT[s], SLOT_QCOL[s]
                for mm in range(4):
                    hp = 4 * g + mm
                    ots = [otp.tile([DK + 1, 512], F32, tag=t,
                                    name=f"ot{t}_{s}_{mm}")
                           for t in ("e", "o")]
                    for i in range(n):
                        l = i - (n - 4)       # >= 0 on causal-diagonal tiles
                        off = 128 * l if l > 0 else 0
                        st = stp.tile([128, 1024], F32, tag="st")
                        for hh in range(2):
                            nc.tensor.matmul(
                                st[:, hh * 512 + off:hh * 512 + 512],
                                kt[hh * 64:hh * 64 + 64, hp,
                                   i * 128:(i + 1) * 128],
                                qt[hh * 64:hh * 64 + 64, hp,
                                   qcol + off:qcol + 512],
                                start=True, stop=True,
                            )
                        e = epool.tile([128, 1024], BF16, tag="e")
                        if l <= 0:
                            nc.scalar.activation(e[:], st[:], EXP,
                                                 scale=float(SCALE))
                        else:
                            for hh in range(2):
                                csl = slice(hh * 512 + off, hh * 512 + 512)
                                nc.scalar.activation(e[:, csl], st[:, csl],
                                                     EXP, scale=float(SCALE))
                        if pend is not None:
                            emit_av(*pend)
                            emit_outproj()
                        if l >= 0:
                            for hh in range(2):
                                o = hh * 512 + off
                                nc.gpsimd.tensor_mul(e[:, o:o + 128],
                                                     e[:, o:o + 128], mk[:])
                        pend = (s, mm, hp, i, n, off, e, ots)
            if pend is not None:
                emit_av(*pend)
            while deferred:
                emit_outproj()
    nc.compile()
    return nc


def _get(name, builder, *args):
    if name not in _CACHE:
        _CACHE[name] = builder(*args)
    return _CACHE[name]


def _strip_mask01():
    # m01[p, g] = 1 where element (k = p, q = g) of the boundary strip is
    # causally valid (g >= p), else 0.
    p = np.arange(128)[:, None]
    g = np.arange(128)[None, :]
    return (g >= p).astype(NPBF)


def _head(p, m, hh):
    """Head index served by head-pair slot m (0-7), lane hh, on parity p."""
    hg = (m // 4) ^ p
    return 8 * hg + 2 * (m % 4) + hh


def _kernel_numpy(q, k, v, mask, Wq, bq, Wk, bk, Wv, bv, Wo, bo):
    """Slow fp32 fallback for non-causal masks (never hit by the harness)."""
    def proj(x, W, b):
        y = x.reshape(B * S, D) @ np.asarray(W, np.float32).T + b
        return y.reshape(B, S, H, DK).transpose(0, 2, 1, 3)
    qh = proj(q, Wq, np.asarray(bq, np.float32))
    kh = proj(k, Wk, np.asarray(bk, np.float32))
    vh = proj(v, Wv, np.asarray(bv, np.float32))
    m = np.broadcast_to(np.asarray(mask) != 0, (1, 1, S, S))
    out = np.empty((B, H, S, DK), np.float32)
    for b in range(B):
        for h in range(H):
            s = (qh[b, h] @ kh[b, h].T) * SCALE
            s = np.where(m[0, 0], s, -1e30)
            s -= s.max(-1, keepdims=True)
            e = np.exp(s)
            out[b, h] = (e @ vh[b, h]) / e.sum(-1, keepdims=True)
    A = out.transpose(0, 2, 1, 3).reshape(B * S, D)
    y = A @ np.asarray(Wo, np.float32).T + np.asarray(bo, np.float32)
    return y.reshape(B, S, D)


def kernel(q, k, v, mask, Wq, bq, Wk, bk, Wv, bv, Wo, bo):
    q = np.asarray(q, dtype=np.float32)
    k = np.asarray(k, dtype=np.float32)
    v = np.asarray(v, dtype=np.float32)
    m2 = np.asarray(mask).reshape(S, S)
    causal = bool(np.array_equal(m2 != 0, np.tril(np.ones((S, S), bool))))
    if not causal:
        return _kernel_numpy(q, k, v, mask, Wq, bq, Wk, bk, Wv, bv, Wo, bo)
    cores = list(range(NCORES))

    # ---------------- L1: QKV projections (row-sharded) ----------------
    nc1 = _get("proj", _build_proj)
    xqT = np.ascontiguousarray(q.reshape(B * S, D).T.astype(NPBF))   # [D, B*S]
    xkT = np.ascontiguousarray(k.reshape(B * S, D).T.astype(NPBF))
    xvT = np.ascontiguousarray(v.reshape(B * S, D).T.astype(NPBF))
    wqT = np.ascontiguousarray(np.asarray(Wq, np.float32).T.astype(NPBF))
    wkT = np.ascontiguousarray(np.asarray(Wk, np.float32).T.astype(NPBF))
    wvT = np.ascontiguousarray(np.asarray(Wv, np.float32).T.astype(NPBF))
    bqt = np.ascontiguousarray(np.asarray(bq, np.float32).reshape(D // 128, 128).T)
    bkt = np.ascontiguousarray(np.asarray(bk, np.float32).reshape(D // 128, 128).T)
    bvt = np.ascontiguousarray(np.asarray(bv, np.float32).reshape(D // 128, 128).T)
    in1 = [
        {
            "xq": np.ascontiguousarray(xqT[:, c * RPC:(c + 1) * RPC]),
            "xk": np.ascontiguousarray(xkT[:, c * RPC:(c + 1) * RPC]),
            "xv": np.ascontiguousarray(xvT[:, c * RPC:(c + 1) * RPC]),
            "wq": wqT, "wk": wkT, "wv": wvT,
            "bq": bqt, "bk": bkt, "bv": bvt,
        }
        for c in cores
    ]
    r1 = run_bass_kernel_spmd(nc1, in1, core_ids=cores)
    QT = np.concatenate([r1.results[c]["qt"] for c in cores], axis=1)  # [D, B*S]
    KT = np.concatenate([r1.results[c]["kt"] for c in cores], axis=1)
    VT = np.concatenate([r1.results[c]["vt"] for c in cores], axis=1)

    # ---------------- L2: fused attention + out-proj --------------------
    nc2 = _get("attn", _build_attn)
    Qh = np.asarray(QT).reshape(H, DK, B, S)      # [h, d, b, s]
    Kh = np.asarray(KT).reshape(H, DK, B, S)
    Vh = np.asarray(VT).reshape(H, DK, B, S)
    WoT = np.ascontiguousarray(np.asarray(Wo, np.float32).T.astype(NPBF))
    WoT3 = WoT.reshape(H, DK, D)                  # [h, d, of]
    m01 = _strip_mask01()
    JA, JB = (0, 1), (3, 2)                       # per head-pair group

    in2 = []
    for c in cores:
        b, p = c // 2, c % 2
        qt = np.zeros((8, 128, 1024), NPBF)
        ktm = np.zeros((8, 128, 2048), NPBF)
        vpm = np.zeros((8, 128, 2, 16, DK + 1), NPBF)
        wom = np.empty((8, 128, D), NPBF)
        for m in range(8):
            g = m // 4
            kl, nt = GRP_KL[g], GRP_NT[g]
            for hh in range(2):
                h = _head(p, m, hh)
                rows = slice(hh * 64, hh * 64 + 64)
                qt[m, rows, 0:512] = Qh[h, :, b, 512 * JA[g]:512 * JA[g] + 512]
                qt[m, rows, 512:1024] = Qh[h, :, b, 512 * JB[g]:512 * JB[g] + 512]
                ktm[m, rows, 0:kl] = Kh[h, :, b, 0:kl]
                vb = Vh[h, :, b, 0:kl].reshape(DK, nt, 128)
                vpm[m, :, hh, 0:nt, 0:DK] = vb.transpose(2, 1, 0)
                vpm[m, :, hh, 0:nt, DK] = 1.0
                wom[m, rows, :] = WoT3[h]
        in2.append({"qt": qt, "kt": ktm, "vp": vpm, "wo": wom, "m01": m01})
    r2 = run_bass_kernel_spmd(nc2, in2, core_ids=cores)

    # ---------------- host: sum head-group partials, add bias -----------
    y = np.empty((B, S, D), np.float32)
    acc = np.zeros((B, 4, D, 512), np.float32)    # [b, j, of, q]
    for c in cores:
        b = c // 2
        yp = np.asarray(r2.results[c]["y"])       # [4, 8, 128, 512]
        for s, (g, j) in enumerate(SLOT_GJ):
            acc[b, j] += yp[s].reshape(D, 512)
    for b in range(B):
        for j in range(4):
            y[b, 512 * j:512 * (j + 1), :] = acc[b, j].T
    y += np.asarray(bo, np.float32)[None, None, :]
    return y
